# revision 30
# baseline (speedup 1.0000x reference)
"""Trainium2 Bass kernel v3: v2 + fine-grained tail scheduling.

Changes vs v2:
  - prev-pair tail (norm + final projection + store) is a work queue spread
    across the next pair's attention steps, per-(head,qch) granularity:
    recip -> half-broadcast [64,512] -> 4 mults -> fp ct-pairs, so PE's fp
    matmuls become ready step by step instead of waiting the full norm.
  - wo tiles allocated from the x-chunk pool: the WAR dependency on old x
    slots delays the wo DMAs out of the startup window where they stole
    bandwidth from the projection feed.
  - off DMA gated behind a 1-element memset (WAW dep) emitted after K proj.
  - K-proj drains split DVE/ACT so KT jt0 is ready sooner for pair-0 scores.
  - per-head rc/rcb tiles decouple the two heads' norm chains.
"""

import numpy as np
import ml_dtypes

import concourse.bass as bass
import concourse.bacc as bacc
import concourse.tile as tile
from concourse import mybir
from concourse.bass_utils import run_bass_kernel_spmd

F32 = mybir.dt.float32
BF16 = mybir.dt.bfloat16

B, S, D, H = 4, 1024, 1024, 16
d_head = D // H
HPC = 8
JC = HPC * d_head

_cached = {}


def build_program(use_mask: bool, loop_n=None, lag=4, last_lag=2, per_step=6):
    nc = bacc.Bacc(None, target_bir_lowering=False, debug=False)

    xqT = nc.dram_tensor("xqT", [D, S], BF16, kind="ExternalInput").ap()
    xkT = nc.dram_tensor("xkT", [D, S], BF16, kind="ExternalInput").ap()
    xvT = nc.dram_tensor("xvT", [D, S], BF16, kind="ExternalInput").ap()
    wqT = nc.dram_tensor("wqT", [D, JC], BF16, kind="ExternalInput").ap()
    wkT = nc.dram_tensor("wkT", [D, JC], BF16, kind="ExternalInput").ap()
    wvT = nc.dram_tensor("wvT", [D, JC], BF16, kind="ExternalInput").ap()
    bq_col = nc.dram_tensor("bq_col", [128, 4], F32, kind="ExternalInput").ap()
    woT = nc.dram_tensor("woT", [D, D], BF16, kind="ExternalInput").ap()
    off = nc.dram_tensor("off", [128, 4096], BF16, kind="ExternalInput").ap()
    if use_mask:
        pen = nc.dram_tensor("pen", [S, S], BF16, kind="ExternalInput").ap()
    out = nc.dram_tensor("out", [JC, D], F32, kind="ExternalOutput").ap()

    with tile.TileContext(nc) as tc:
        with (
            tc.tile_pool(name="xp", bufs=10) as xp,
            tc.tile_pool(name="wp", bufs=8) as wp,
            tc.tile_pool(name="qt", bufs=4) as qt_p,
            tc.tile_pool(name="kt", bufs=4) as kt_p,
            tc.tile_pool(name="va", bufs=8) as va_p,
            tc.tile_pool(name="wT", bufs=10) as wT_p,
            tc.tile_pool(name="lh", bufs=2) as lh_p,
            tc.tile_pool(name="outp", bufs=2) as outp,
            tc.tile_pool(name="small", bufs=8) as smallp,
            tc.tile_pool(name="pen", bufs=8) as pen_p,
            tc.tile_pool(name="psA", bufs=2, space="PSUM") as psA,
            tc.tile_pool(name="psB", bufs=4, space="PSUM") as psB,
        ):
            from contextlib import ExitStack
            _stk = ExitStack()
            if loop_n is not None:
                _stk.enter_context(tc.For_i(0, loop_n))

            warm = smallp.tile([1, 8], F32, tag="warm", bufs=1)
            nc.vector.memset(warm[:], 0.0)
            nc.scalar.activation(warm[:], warm[:],
                                 mybir.ActivationFunctionType.Exp)

            def load_x(dram, first_on_sp=False):
                ts = []
                for i in range(8):
                    t = xp.tile([128, 1024], BF16, tag="x", name="x")
                    eng = nc.sync if (first_on_sp and i == 0) else nc.gpsimd
                    eng.dma_start(t[:], dram[i * 128:(i + 1) * 128, :])
                    ts.append(t)
                return ts

            def load_w(dram):
                ts = []
                for i in range(8):
                    t = wp.tile([128, 512], BF16, tag="w", name="w")
                    nc.sync.dma_start(t[:], dram[i * 128:(i + 1) * 128, :])
                    ts.append(t)
                return lambda dt: ts[dt][:]

            wt_q = load_w(wqT)
            xt_q = load_x(xqT)
            wt_k = load_w(wkT)
            xt_k = load_x(xkT)
            wt_v = load_w(wvT)
            xt_v = load_x(xvT)

            bq_sb = smallp.tile([128, 4], F32, tag="bias", bufs=1)
            nc.gpsimd.dma_start(bq_sb[:], bq_col[:])

            pen_t = None
            if use_mask:
                pts = []
                for i in range(8):
                    t = pen_p.tile([128, 1024], BF16, tag="pen", name="pen")
                    nc.gpsimd.dma_start(t[:], pen[i * 128:(i + 1) * 128, :])
                    pts.append(t)
                pen_t = lambda kt: pts[kt]

            def proj_qk(wt, xts, dst_pool, drain):
                bigs = [psA.tile([128, 1024], F32, tag="sc", name="prj")
                        for _ in range(2)]
                sms = [psB.tile([128, 512], F32, tag="ps1", name="prj")
                       for _ in range(4)]

                def pview(jt, st):
                    if jt < 2:
                        return bigs[jt][:, st * 512:(st + 1) * 512]
                    return sms[(jt - 2) * 2 + st][:]

                for dt in range(8):
                    for jt in range(4):
                        for st in range(2):
                            nc.tensor.matmul(
                                pview(jt, st),
                                lhsT=wt(dt)[:, jt * 128:(jt + 1) * 128],
                                rhs=xts[dt][:, st * 512:(st + 1) * 512],
                                start=(dt == 0), stop=(dt == 7),
                            )
                dst = [dst_pool.tile([128, S], BF16, tag="dst", name="dst")
                       for _ in range(4)]
                for jt in range(4):
                    for st in range(2):
                        drain(dst[jt][:, st * 512:(st + 1) * 512],
                              pview(jt, st), jt)
                return dst

            def q_drain(dst, ps, jt):
                # split DVE/ACT so K proj's psum-slot reuse isn't serialized
                # behind one engine's drain queue
                if jt in (0, 1):
                    nc.vector.tensor_scalar_add(dst, ps, bq_sb[:, jt:jt + 1])
                else:
                    nc.scalar.activation(dst, ps,
                                         mybir.ActivationFunctionType.Identity,
                                         bias=bq_sb[:, jt:jt + 1])

            def k_drain(dst, ps, jt):
                # jt0/jt1 on ACT (idle then) so KT is ready for early pairs
                if jt in (0, 1):
                    nc.scalar.activation(dst, ps,
                                         mybir.ActivationFunctionType.Copy)
                else:
                    nc.vector.tensor_copy(dst, ps)

            QT = proj_qk(wt_q, xt_q, qt_p, q_drain)
            KT = proj_qk(wt_k, xt_k, kt_p, k_drain)

            # wo from the x pool: WAR dep on old x slots delays these DMAs
            # past the projection feed window
            wo_ts = []
            for i in range(8):
                t = xp.tile([128, 1024], BF16, tag="x", name="wo")
                nc.sync.dma_start(t[:], woT[i * 128:(i + 1) * 128, :])
                wo_ts.append(t)
            off_sb = smallp.tile([128, 4096], BF16, tag="off", bufs=1)
            # true-data WAW gate: the copy reads KT (ready ~K-proj end), so
            # the off DMA can't steal bandwidth from the projection feed
            nc.vector.tensor_copy(off_sb[0:1, 0:1], KT[0][0:1, 0:1])
            nc.sync.dma_start(off_sb[:], off[:])

            # ---- V projection -> V_aug [s, 8*65] (65th col per head = 1.0)
            VA = []
            for st in range(8):
                ps = psB.tile([128, 512], F32, tag="ps1", name="vprj")
                for dt in range(8):
                    nc.tensor.matmul(
                        ps[:],
                        lhsT=xt_v[dt][:, st * 128:(st + 1) * 128],
                        rhs=wt_v(dt),
                        start=(dt == 0), stop=(dt == 7),
                    )
                va = va_p.tile([128, 8 * 65], BF16, name="va")
                vv = va[:].rearrange("p (h c) -> p h c", h=8)
                nc.vector.memset(vv[:, :, 64:65], 1.0)
                nc.vector.tensor_copy(
                    vv[:, :, 0:64],
                    ps[:].rearrange("p (h c) -> p h c", h=8))
                VA.append(va)

            def QT_perm(hl, qch):
                tile_ = QT[hl // 2]
                po = (hl % 2) * 64
                ap = tile_[po:po + 64, :].rearrange("p (q s) -> p s q", s=16)
                return ap[:, qch * 8:(qch + 1) * 8, :]

            def KT_ap(hl, kt):
                tile_ = KT[hl // 2]
                po = (hl % 2) * 64
                return tile_[po:po + 64, kt * 128:(kt + 1) * 128]

            # per-head rc/rcb; rows 1-63 of rc only feed the bcast AP
            rcs, rcbs = [], []
            for hloc in range(2):
                rc = smallp.tile([64, 1024], F32, tag=f"rc{hloc}", bufs=1,
                                 name="rc")
                nc.vector.memset(rc[:], 1.0)
                rcs.append(rc)
                rcb = smallp.tile([64, 1024], F32, tag=f"rcb{hloc}", bufs=1,
                                  name="rcb")
                rcbs.append(rcb)

            def make_tail(pp, ppv, fine=False):
                """Work queue for pair pp's normalize + final projection."""
                W = []
                st8 = {}

                def alloc_lh():
                    st8["lh"] = lh_p.tile([128, 1024], BF16, name="lh")

                def recip(hloc, qch):
                    i = hloc + 2 * qch
                    nc.vector.reciprocal(
                        rcs[hloc][0:1, qch * 512:(qch + 1) * 512],
                        ppv[i][64:65, :])

                def bcast(hloc, qch):
                    nc.gpsimd.partition_broadcast(
                        rcbs[hloc][:, qch * 512:(qch + 1) * 512],
                        rcs[hloc][:, qch * 512:(qch + 1) * 512])

                def mult(hloc, qch, par):
                    i = hloc + 2 * qch
                    lh = st8["lh"]
                    src = ppv[i][0:64, :].rearrange("p (s q) -> p s q", s=8)
                    rcv = rcbs[hloc][:].rearrange("p (s q) -> p s q", s=16)
                    dst = lh[par * 64:par * 64 + 64, :].rearrange(
                        "p (c m) -> p c m", c=8
                    )[:, qch * 4:(qch + 1) * 4, hloc * 64:(hloc + 1) * 64]
                    nc.vector.tensor_tensor(
                        dst, src[:, par::2, :],
                        rcv[:, qch * 8 + par:qch * 8 + 8:2, :],
                        op=mybir.AluOpType.mult)

                def fp(ot, j):
                    if j == 0:
                        st8[f"fp{ot}"] = psB.tile([128, 512], F32, tag="ps1",
                                                  name="fp")
                    for ct in (2 * j, 2 * j + 1):
                        nc.tensor.matmul(
                            st8[f"fp{ot}"][:],
                            lhsT=st8["lh"][:, ct * 128:(ct + 1) * 128],
                            rhs=wo_ts[ct][:, ot * 512:(ot + 1) * 512],
                            start=(ct == 0), stop=(ct == 7))

                def drain(ot, quarters=1):
                    if ot == 0:
                        st8["ob"] = outp.tile([128, 1024], F32, name="ob")
                    ob = st8["ob"]
                    qw = 512 // quarters
                    for qi in range(quarters):
                        lo, hi = ot * 512 + qi * qw, ot * 512 + (qi + 1) * qw
                        nc.vector.tensor_tensor(
                            ob[:, lo:hi], st8[f"fp{ot}"][:, qi * qw:(qi + 1) * qw],
                            off_sb[:, pp * 1024 + lo:pp * 1024 + hi],
                            op=mybir.AluOpType.add)
                        nc.sync.dma_start(
                            out[pp * 128:(pp + 1) * 128, lo:hi], ob[:, lo:hi])

                W.append(alloc_lh)
                for qch in range(2):
                    for hloc in range(2):
                        W.append(lambda h=hloc, q=qch: recip(h, q))
                    for hloc in range(2):
                        W.append(lambda h=hloc, q=qch: bcast(h, q))
                    for hloc in range(2):
                        for par in range(2):
                            W.append(lambda h=hloc, q=qch, p_=par:
                                     mult(h, q, p_))
                for ot in range(2):
                    for j in range(4):
                        W.append(lambda o=ot, j_=j: fp(o, j_))
                    W.append(lambda o=ot: drain(o))
                return W

            def attention(p, tail, lag_):
                # lag_ may be (lag_q0, lag_q1): qch0's PV runs ahead so its
                # norm chain can overlap the remaining qch1 PV steps
                lag0, lag1 = lag_ if isinstance(lag_, tuple) else (lag_, lag_)
                hA, hB = 2 * p, 2 * p + 1
                pv = {}
                wstash = {}
                consumed = 0
                for step in range(8 + lag1):
                    want = min(len(tail), (step + 1) * per_step)
                    while consumed < want:
                        tail[consumed]()
                        consumed += 1
                    if step < 8:
                        kt = step
                        scA = psA.tile([128, 1024], F32, tag="sc")
                        scB = psA.tile([128, 1024], F32, tag="sc")
                        for qch in range(2):
                            nc.tensor.matmul(
                                scA[:, qch * 512:(qch + 1) * 512],
                                lhsT=KT_ap(hA, kt), rhs=QT_perm(hA, qch),
                                start=True, stop=True)
                            nc.tensor.matmul(
                                scB[:, qch * 512:(qch + 1) * 512],
                                lhsT=KT_ap(hB, kt), rhs=QT_perm(hB, qch),
                                start=True, stop=True)
                        wA = wT_p.tile([128, 1024], BF16, tag="wT")
                        wB = wT_p.tile([128, 1024], BF16, tag="wT")
                        nc.scalar.activation(wA[:], scA[:],
                                             mybir.ActivationFunctionType.Exp,
                                             scale=0.125)
                        nc.scalar.activation(wB[:], scB[:],
                                             mybir.ActivationFunctionType.Exp,
                                             scale=0.125)
                        if use_mask:
                            pap = pen_t(kt).rearrange("p (q s) -> p s q", s=16)
                            for w_ in (wA, wB):
                                nc.vector.tensor_tensor(
                                    w_[:].rearrange("p (s q) -> p s q", s=16),
                                    w_[:].rearrange("p (s q) -> p s q", s=16),
                                    pap, op=mybir.AluOpType.mult)
                        wstash[kt] = (wA, wB)
                    for qch, lg in ((0, lag0), (1, lag1)):
                        if not (lg <= step < 8 + lg):
                            continue
                        kt = step - lg
                        wA, wB = wstash[kt] if qch == 0 else wstash.pop(kt)
                        for i, (hl, wt_) in enumerate([(hA, wA), (hB, wB)]):
                            i += 2 * qch
                            if kt == 0:
                                pv[i] = psB.tile([65, 512], F32, tag="ps1",
                                                 name="pv")
                            nc.tensor.matmul(
                                pv[i][:],
                                lhsT=VA[kt][:, hl * 65:hl * 65 + 65],
                                rhs=wt_[:, qch * 512:(qch + 1) * 512],
                                start=(kt == 0), stop=(kt == 7))
                while consumed < len(tail):
                    tail[consumed]()
                    consumed += 1
                return pv

            pending = None
            for p in range(4):
                tail = [] if pending is None else make_tail(*pending)
                pv = attention(p, tail, lag if p < 3 else last_lag)
                pending = (p, pv)
            for fn in make_tail(*pending, fine=True):
                fn()
            _stk.close()

    nc.compile()
    return nc


def _bf16(a):
    return np.asarray(a, np.float32).astype(ml_dtypes.bfloat16)


def make_in_maps(query, key, value, mask, Wq, bq, Wk, bk, Wv, bv, Wo,
                 pen_b=None):
    woT = _bf16(Wo.T)
    Wo32 = np.asarray(Wo, np.float32)
    maps = []
    for c in range(8):
        b, hf = c // 2, c % 2
        sl = slice(hf * JC, (hf + 1) * JC)
        bv_sl = np.asarray(bv, np.float32)[sl]
        offvec = np.stack([
            np.tile(bv_sl[h * 64:(h + 1) * 64], 16) @ Wo32.T
            for h in range(8)
        ])  # [8, 1024]
        off_arr = np.empty((128, 4096), np.float32)
        for p in range(4):
            for part in range(128):
                off_arr[part, p * 1024:(p + 1) * 1024] = \
                    offvec[2 * p + part // 64]
        m = {
            "xqT": _bf16(query[b].T),
            "xkT": _bf16(key[b].T),
            "xvT": _bf16(value[b].T),
            "wqT": _bf16(Wq[sl].T),
            "wkT": _bf16(Wk[sl].T),
            "wvT": _bf16(Wv[sl].T),
            "bq_col": np.ascontiguousarray(
                np.asarray(bq, np.float32)[sl].reshape(4, 128).T),
            "woT": woT,
            "off": off_arr.astype(ml_dtypes.bfloat16),
        }
        if pen_b is not None:
            m["pen"] = _bf16(pen_b[b])
        maps.append(m)
    return maps


def kernel(query, key, value, mask, Wq, bq, Wk, bk, Wv, bv, Wo):
    query = np.asarray(query, np.float32)
    key = np.asarray(key, np.float32)
    value = np.asarray(value, np.float32)
    mask = np.asarray(mask, np.float32)

    m2d = mask[0]
    mm = np.stack([m2d[b] @ m2d[b].T for b in range(B)])
    use_mask = bool((mm == 0).any())
    pen_b = None
    if use_mask:
        pen_b = np.where(mm == 0, np.float32(0.0), np.float32(1.0))
        pen_b = np.ascontiguousarray(pen_b, np.float32)

    if use_mask not in _cached:
        _cached[use_mask] = build_program(use_mask)
    nc = _cached[use_mask]

    in_maps = make_in_maps(query, key, value, mask,
                           np.asarray(Wq, np.float32), np.asarray(bq, np.float32),
                           np.asarray(Wk, np.float32), np.asarray(bk, np.float32),
                           np.asarray(Wv, np.float32), np.asarray(bv, np.float32),
                           np.asarray(Wo, np.float32), pen_b)
    res = run_bass_kernel_spmd(nc, in_maps, list(range(8)))

    out = np.empty((B, S, D), np.float32)
    for c in range(8):
        b, hf = c // 2, c % 2
        out[b, hf * JC:(hf + 1) * JC, :] = res.results[c]["out"]
    return out
